# revision 1
# baseline (speedup 1.0000x reference)
"""Trainium2 kernel for nn_Dense_Q_MulIn1Out_Conv1D.

The reference "quantum conv" circuit is linear in the state vector: three
RY-rotation layers interleaved with a fixed 512x512 orthogonal entangler.
The whole circuit therefore collapses to one matrix M (512x512), and since
the encoded state has only its first 128 amplitudes nonzero, the <Z> readout
reduces to a quadratic form with a fixed symmetric 128x128 matrix A:

    out[n] = (v_n^T A v_n) / (||v_n||^2 + 1e-12)

where v_n is the (unnormalized) im2col patch of x (C=16 channels x K=8 taps,
channel-major).  A = Md^T Z Md with Md = M[:, :128], Z = diag(+1 x256, -1 x256).

Host side: build A (float64) from entangle_matrix/theta, permute it to
k-major patch order so the on-device im2col is 8 shifted row-block copies.
Device side (per core, 2 of 16 batches): build V [128, 4096] per batch by
DMA, Y = A @ V on TensorE (fp32r), P1 = V*Y, P2 = V*V elementwise, then
ones-vector matmuls reduce partitions to num/den rows of a [16, 512] PSUM
tile; final reciprocal-multiply and one 32KB store.
"""

import numpy as np

_DIM = 512
_D = 128
_K = 8
_C = 16
_NQ = 9
_B = 16
_L = 4096
_L_OUT = _L - _K + 1  # 4089
_N_CORES = 8
_B_PER_CORE = _B // _N_CORES  # 2
_NCHUNK = 8  # 512-column chunks per batch
_CHUNK = 512

# k-major patch permutation: new index p = k*16 + c  <->  old index c*8 + k
_PERM = np.array([(p % _C) * _K + (p // _C) for p in range(_D)])


def _apply_ry_layer(psi, angles):
    # psi [N, DIM] float64; matches reference._apply_ry_layer
    for q in range(_NQ):
        half = angles[q] * 0.5
        c, s = np.cos(half), np.sin(half)
        left = 2 ** q
        p = psi.reshape(-1, left, 2, _DIM // (2 ** (q + 1)))
        a, b = p[:, :, 0, :].copy(), p[:, :, 1, :].copy()
        psi = np.stack([c * a - s * b, s * a + c * b], axis=2).reshape(-1, _DIM)
    return psi


def _build_amat(entangle_matrix, theta):
    """Collapse the circuit to the k-major-permuted 128x128 quadratic form."""
    U = np.asarray(entangle_matrix, dtype=np.float64)
    th = np.asarray(theta, dtype=np.float64)
    psi = np.eye(_DIM, dtype=np.float64)
    for l in range(th.shape[0]):
        psi = _apply_ry_layer(psi, th[l])
        psi = psi @ U.T
    M = psi.T  # state map: s -> M s
    z = np.concatenate([np.ones(_DIM // 2), -np.ones(_DIM // 2)])
    Md = M[:, :_D]
    A = Md.T @ (z[:, None] * Md)
    A_km = A[np.ix_(_PERM, _PERM)]
    return np.ascontiguousarray(A_km, dtype=np.float32)


_NC_CACHE = {}


def _build_nc():
    import concourse.tile as tile
    from concourse import bacc, mybir

    F32 = mybir.dt.float32
    F32R = mybir.dt.float32r
    AF = mybir.ActivationFunctionType

    nc = bacc.Bacc(
        "TRN2",
        target_bir_lowering=False,
        debug=False,
        num_devices=_N_CORES,
    )
    ngl = _B_PER_CORE * _NCHUNK  # 16 global chunks
    # flat x + 8 pad elements so the im2col window never reads out of bounds
    x = nc.dram_tensor(
        "x", [_B_PER_CORE * _C * _L + _K], F32, kind="ExternalInput"
    ).ap()
    # consts = [A_km | T2] with T2 [128, 64]: single ones-column at col 32.
    # A 32-wide window T2[:, 32-m : 64-m] is a selector matrix whose matmul
    # sums all partitions into output partition m (ones at in-window col m).
    consts = nc.dram_tensor(
        "consts", [_D, _D + 96], F32, kind="ExternalInput"
    ).ap()
    out = nc.dram_tensor(
        "out", [_B_PER_CORE * _NCHUNK, _CHUNK], F32, kind="ExternalOutput"
    ).ap()

    with tile.TileContext(nc) as tc:
        from contextlib import ExitStack

        with ExitStack() as ctx:
            const_pool = ctx.enter_context(tc.tile_pool(name="const", bufs=1))
            v_pool = ctx.enter_context(tc.tile_pool(name="v", bufs=2))
            p_pool = ctx.enter_context(tc.tile_pool(name="p", bufs=2))
            y_pool = ctx.enter_context(tc.tile_pool(name="y", bufs=2, space="PSUM"))
            red_pool = ctx.enter_context(tc.tile_pool(name="red", bufs=1, space="PSUM"))
            o_pool = ctx.enter_context(tc.tile_pool(name="o", bufs=1))

            c_sb = const_pool.tile([_D, _D + 96], F32)
            nc.scalar.dma_start(c_sb[:].bitcast(F32R), consts[:].bitcast(F32R))
            a_sb = c_sb[:, :_D]
            t2 = c_sb[:, _D:]

            def sel_num(g):
                # ones at within-window col g -> output partition g (num)
                return t2[:, 48 - g : 96 - g].bitcast(F32R)

            def sel_den(g):
                # ones at col 32+g -> output partition 32+g (den; 32-aligned
                # so the epilogue's partition-offset reads are legal)
                return t2[:, 16 - g : 64 - g].bitcast(F32R)

            # num rows 0..15, den rows 32..47, one PSUM bank total
            red = red_pool.tile([48, _CHUNK], F32)

            from bass_rust import AP as RawAP

            # V free size is _L+1 so its partition pitch (4097) can't be
            # coalesced with the 4096-element column runs by the DMA AP
            # balancer (a flat run crossing SBUF partitions is invalid).
            _LV = _L + 1
            _Q = 1024  # quarter width: DMA piece + y-tile width
            vs = []
            for b in range(_B_PER_CORE):
                v = v_pool.tile([_D, _LV], F32, tag="v")
                vs.append(v)
                # im2col in 4 column-quarters, alternating the two HWDGE
                # rings (sync / scalar) so all 16 SDMA engines run.
                # dst partition (k*16+c), col n <- x[b, c, n+k]; cols >=
                # L_OUT pick up neighboring-channel garbage (host discards).
                for q in range(4):
                    dst = v[:, q * _Q : (q + 1) * _Q].bitcast(F32R)
                    srcap = RawAP(
                        tensor=x.tensor, offset=b * _C * _L + q * _Q,
                        ap=[[1, _K], [_L, _C], [1, _Q]],
                    ).bitcast(F32R)
                    eng = nc.sync if q % 2 == 0 else nc.scalar
                    eng.dma_start(dst, srcap)

            mm_i = 0  # running index over all 64 reduction matmuls
            for b in range(_B_PER_CORE):
                v = vs[b]
                for h in range(2):  # 2048-wide halves for the squares
                    p2 = p_pool.tile([_D, 2 * _Q], F32, tag="p2")
                    nc.scalar.activation(
                        p2[:].bitcast(F32R),
                        v[:, h * 2 * _Q : (h + 1) * 2 * _Q],
                        AF.Square,
                    )
                    for qq in range(2):  # 1024-wide y tiles
                        base = h * 2 * _Q + qq * _Q
                        g0 = b * _NCHUNK + (base // _CHUNK)
                        y = y_pool.tile([_D, _Q], F32)
                        for s in range(2):
                            nc.tensor.matmul(
                                y[:, s * _CHUNK : (s + 1) * _CHUNK],
                                a_sb.bitcast(F32R),
                                v[:, base + s * _CHUNK : base + (s + 1) * _CHUNK]
                                .bitcast(F32R),
                                start=True, stop=True,
                            )
                        p1 = p_pool.tile([_D, _Q], F32, tag="p1")
                        nc.vector.tensor_mul(
                            p1[:].bitcast(F32R), v[:, base : base + _Q], y[:]
                        )
                        for s in range(2):
                            g = g0 + s
                            sl = slice(s * _CHUNK, (s + 1) * _CHUNK)
                            nc.tensor.matmul(
                                red[:], sel_num(g), p1[:, sl].bitcast(F32R),
                                start=(mm_i == 0), stop=(mm_i == 63),
                                skip_group_check=True,
                            )
                            mm_i += 1
                            sl2 = slice(qq * _Q + s * _CHUNK,
                                        qq * _Q + (s + 1) * _CHUNK)
                            nc.tensor.matmul(
                                red[:], sel_den(g), p2[:, sl2].bitcast(F32R),
                                start=(mm_i == 0), stop=(mm_i == 63),
                                skip_group_check=True,
                            )
                            mm_i += 1

            den_sb = o_pool.tile([16, _CHUNK], F32, tag="den")
            nc.scalar.activation(den_sb[:], red[32:48, :], AF.Copy, bias=1e-12)
            rden = o_pool.tile([16, _CHUNK], F32, tag="rden")
            nc.vector.reciprocal_approx_fast(rden[:], den_sb[:])
            out_sb = o_pool.tile([16, _CHUNK], F32, tag="outsb")
            nc.vector.tensor_mul(out_sb[:], red[0:16, :], rden[:])
            nc.sync.dma_start(out[:], out_sb[:])

    nc.compile()
    return nc


def get_nc():
    if "nc" not in _NC_CACHE:
        _NC_CACHE["nc"] = _build_nc()
    return _NC_CACHE["nc"]


def kernel(x, entangle_matrix, theta, _trace=False, **trace_kwargs):
    from concourse.bass_utils import run_bass_kernel_spmd

    x = np.asarray(x, dtype=np.float32)
    amat = _build_amat(entangle_matrix, theta)
    # T2: single ones-column at col 32; sliding 32-wide windows of T2 give
    # every selector matrix (ones exactly at within-block column g).
    t2 = np.zeros((_D, 96), dtype=np.float32)
    t2[:, 48] = 1.0
    consts = np.ascontiguousarray(np.concatenate([amat, t2], axis=1))

    nc = get_nc()
    pad = np.zeros(_K, dtype=np.float32)
    in_maps = [
        {
            "x": np.concatenate(
                [x[i * _B_PER_CORE : (i + 1) * _B_PER_CORE].reshape(-1), pad]
            ),
            "consts": consts,
        }
        for i in range(_N_CORES)
    ]
    res = run_bass_kernel_spmd(
        nc, in_maps, list(range(_N_CORES)), trace=_trace, **trace_kwargs
    )
    outs = []
    for i in range(_N_CORES):
        o = np.asarray(res.results[i]["out"], dtype=np.float32)
        outs.append(o.reshape(_B_PER_CORE, _NCHUNK * _CHUNK)[:, :_L_OUT])
    full = np.concatenate(outs, axis=0).reshape(_B, 1, 1, _L_OUT)
    if _trace:
        kernel._last_results = res
    return full



# revision 10
# speedup vs baseline: 1.5113x; 1.5113x over previous
"""Trainium2 kernel for nn_Dense_Q_MulIn1Out_Conv1D.

The reference "quantum conv" circuit is linear in the state vector, so the
whole circuit collapses to a fixed symmetric 128x128 quadratic form A over
the (unnormalized) im2col patch v_n:

    out[n] = (v_n^T A v_n) / (||v_n||^2 + 1e-12)

Eigendecompose A = Q diag(lam) Q^T and let L = diag(sqrt(|lam|+mu)) Q^T.
With W = L V (V the k-major im2col matrix):

    num[n] = sum_i sign(lam_i)|lam_i|/(|lam_i|+mu) * W[i,n]^2
    den[n] = sum_i 1/(|lam_i|+mu)               * W[i,n]^2   ( = ||v_n||^2 )

so ONE matmul (W = L V, fp16) plus elementwise squares plus ONE weighted
reduction matmul per column chunk produce both numerator and denominator.
The division happens on host.

Per core (2 of 16 batches): V [128, 4096] fp16 per batch is built by
im2col DMA in 1024-column quarters (2KB descriptors round-robin across all
16 SDMA engines; 4KB descriptors would occupy two engine slots each and
only reach 8 engines). Squares alternate between the Scalar and Vector
engines. Reduction matmuls accumulate into one PSUM bank per batch with a
sliding-window selector whose two weight columns land num in partitions
0..7 and den in partitions 8..15.
"""

import numpy as np

_DIM = 512
_D = 128
_K = 8
_C = 16
_NQ = 9
_B = 16
_L = 4096
_L_OUT = _L - _K + 1  # 4089
_N_CORES = 8
_B_PER_CORE = _B // _N_CORES  # 2
_NCHUNK = 8  # 512-column chunks per batch
_CHUNK = 512
_MU = 0.01

# k-major patch permutation: new index p = k*16 + c  <->  old index c*8 + k
_PERM = np.array([(p % _C) * _K + (p // _C) for p in range(_D)])


def _apply_ry_layer(psi, angles):
    # psi [N, DIM] float64; matches reference._apply_ry_layer
    for q in range(_NQ):
        half = angles[q] * 0.5
        c, s = np.cos(half), np.sin(half)
        left = 2 ** q
        p = psi.reshape(-1, left, 2, _DIM // (2 ** (q + 1)))
        a, b = p[:, :, 0, :].copy(), p[:, :, 1, :].copy()
        psi = np.stack([c * a - s * b, s * a + c * b], axis=2).reshape(-1, _DIM)
    return psi


def _build_consts(entangle_matrix, theta):
    """Collapse the circuit to the quadratic form, eigendecompose, and pack
    the fp16 constant block [128, 160] = [LT | Tsel]."""
    U = np.asarray(entangle_matrix, dtype=np.float64)
    th = np.asarray(theta, dtype=np.float64)
    psi = np.eye(_DIM, dtype=np.float64)
    for l in range(th.shape[0]):
        psi = _apply_ry_layer(psi, th[l])
        psi = psi @ U.T
    M = psi.T  # state map: s -> M s
    z = np.concatenate([np.ones(_DIM // 2), -np.ones(_DIM // 2)])
    Md = M[:, :_D]
    A = Md.T @ (z[:, None] * Md)
    A_km = A[np.ix_(_PERM, _PERM)]

    lam, Q = np.linalg.eigh(A_km)
    s = np.sqrt(np.abs(lam) + _MU)
    LT = Q * s[None, :]  # LT[p, i] = L[i, p]
    w_num = np.sign(lam) * np.abs(lam) / (np.abs(lam) + _MU)
    w_den = 1.0 / (np.abs(lam) + _MU)
    # Sliding-window selector: window Tsel[:, 8-gl : 24-gl] puts w_num into
    # output partition gl and w_den into partition 8+gl (gl = 0..7).
    tsel = np.zeros((_D, 32), dtype=np.float64)
    tsel[:, 8] = w_num
    tsel[:, 16] = w_den
    consts = np.concatenate([LT, tsel], axis=1)
    return np.ascontiguousarray(consts, dtype=np.float16)


_NC_CACHE = {}


def _build_nc():
    import concourse.tile as tile
    from concourse import bacc, mybir
    from bass_rust import AP as RawAP

    F32 = mybir.dt.float32
    F16 = mybir.dt.float16
    AF = mybir.ActivationFunctionType

    nc = bacc.Bacc(
        "TRN2",
        target_bir_lowering=False,
        debug=False,
        num_devices=_N_CORES,
    )
    # flat x (fp16) + 8 pad elements so the im2col window never reads OOB
    x = nc.dram_tensor(
        "x", [_B_PER_CORE * _C * _L + _K], F16, kind="ExternalInput"
    ).ap()
    consts = nc.dram_tensor("consts", [_D, 160], F16, kind="ExternalInput").ap()
    # out[b] = [16, 512]: partitions 0..7 num chunks, 8..15 den chunks
    out = nc.dram_tensor(
        "out", [_B_PER_CORE, 16, _CHUNK], F32, kind="ExternalOutput"
    ).ap()

    _Q = 1024  # im2col quarter width: 2KB fp16 descriptors

    with tile.TileContext(nc) as tc:
        from contextlib import ExitStack

        with ExitStack() as ctx:
            const_pool = ctx.enter_context(tc.tile_pool(name="const", bufs=1))
            v_pool = ctx.enter_context(tc.tile_pool(name="v", bufs=2))
            p_pool = ctx.enter_context(tc.tile_pool(name="p", bufs=4))
            w_pool = ctx.enter_context(tc.tile_pool(name="w", bufs=3, space="PSUM"))
            red_pool = ctx.enter_context(tc.tile_pool(name="red", bufs=1, space="PSUM"))
            o_pool = ctx.enter_context(tc.tile_pool(name="o", bufs=1))

            c_sb = const_pool.tile([_D, 160], F16)
            nc.sync.dma_start(c_sb[:], consts[:])
            lt = c_sb[:, :_D]

            def sel(gl):
                # window with w_num at in-window col gl, w_den at col 8+gl
                return c_sb[:, _D + 8 - gl : _D + 24 - gl]

            # im2col: V[b][(k*16+c), n] = x[b, c, n+k], in 1024-col quarters
            vs = []
            for b in range(_B_PER_CORE):
                v = v_pool.tile([_D, _L], F16, tag=f"v{b}")
                vs.append(v)
                for q in range(4):
                    dst = v[:, q * _Q : (q + 1) * _Q]
                    srcap = RawAP(
                        tensor=x.tensor, offset=b * _C * _L + q * _Q,
                        ap=[[1, _K], [_L, _C], [1, _Q]],
                    )
                    nc.sync.dma_start(dst, srcap)

            red0 = red_pool.tile([16, _CHUNK], F32, tag="red0")
            red1 = red_pool.tile([16, _CHUNK], F32, tag="red1")
            reds = [red0, red1]
            ored0 = o_pool.tile([16, _CHUNK], F32, tag="ored0")
            ored1 = o_pool.tile([16, _CHUNK], F32, tag="ored1")
            oreds = [ored0, ored1]

            # Chunk pairs: two 512-col mains into one 2-bank PSUM tile, one
            # 1024-wide Scalar-engine square (TensorTensor cannot read PSUM
            # twice, so DVE squares are off the table; wide Act instructions
            # amortize its per-instruction PSUM-access overhead), two
            # accumulating reduction matmuls.
            ps = [None] * 8

            def emit_main(q):  # q = 0..7, chunks 2q and 2q+1
                b = q // 4
                w2 = w_pool.tile([_D, 2 * _CHUNK], F32)
                p2 = p_pool.tile([_D, 2 * _CHUNK], F16)
                ps[q] = p2
                for h in range(2):
                    gl = (2 * q + h) % _NCHUNK
                    nc.tensor.matmul(
                        w2[:, h * _CHUNK : (h + 1) * _CHUNK], lt,
                        vs[b][:, gl * _CHUNK : (gl + 1) * _CHUNK],
                        start=True, stop=True,
                    )
                nc.scalar.activation(p2[:], w2[:], AF.Square)

            def emit_red(q):
                b = q // 4
                for h in range(2):
                    gl = (2 * q + h) % _NCHUNK
                    nc.tensor.matmul(
                        reds[b][:], sel(gl),
                        ps[q][:, h * _CHUNK : (h + 1) * _CHUNK],
                        start=(gl == 0), stop=(gl == _NCHUNK - 1),
                        skip_group_check=True,
                    )
                if (2 * q + 1) % _NCHUNK == _NCHUNK - 1:
                    # DMA cannot read PSUM; bounce through SBUF on DVE.
                    nc.vector.tensor_scalar_mul(oreds[b][:], reds[b][:], 1.0)
                    nc.sync.dma_start(out[b], oreds[b][:])

            for q in range(8):
                emit_main(q)
                if q >= 2:
                    emit_red(q - 2)
            for q in (6, 7):
                emit_red(q)

    nc.compile()
    return nc


def get_nc():
    if "nc" not in _NC_CACHE:
        _NC_CACHE["nc"] = _build_nc()
    return _NC_CACHE["nc"]


def kernel(x, entangle_matrix, theta, _trace=False, **trace_kwargs):
    from concourse.bass_utils import run_bass_kernel_spmd

    x = np.asarray(x, dtype=np.float32).astype(np.float16)
    consts = _build_consts(entangle_matrix, theta)

    nc = get_nc()
    pad = np.zeros(_K, dtype=np.float16)
    in_maps = [
        {
            "x": np.concatenate(
                [x[i * _B_PER_CORE : (i + 1) * _B_PER_CORE].reshape(-1), pad]
            ),
            "consts": consts,
        }
        for i in range(_N_CORES)
    ]
    res = run_bass_kernel_spmd(
        nc, in_maps, list(range(_N_CORES)), trace=_trace, **trace_kwargs
    )
    outs = []
    for i in range(_N_CORES):
        o = np.asarray(res.results[i]["out"], dtype=np.float32)  # [2, 16, 512]
        num = o[:, :8, :].reshape(_B_PER_CORE, _NCHUNK * _CHUNK)[:, :_L_OUT]
        den = o[:, 8:, :].reshape(_B_PER_CORE, _NCHUNK * _CHUNK)[:, :_L_OUT]
        outs.append(num / den)
    full = np.concatenate(outs, axis=0).reshape(_B, 1, 1, _L_OUT)
    if _trace:
        kernel._last_results = res
    return full
